# revision 18
# baseline (speedup 1.0000x reference)
"""CrossTeacherAttention Trainium2 kernel.

Per batch element b (x as [C=256, N=1024], N=H*W):
  Q = Wq @ Xs + bq  [C,N];  K_t = Wk @ Xt_t + bk  [C,N]
  Vt^T = Xt_t^T @ Wv^T  [N,C]  (bv deferred to the end)
  S_t^T[m,n] = sum_c K_t[c,m] Q[c,n];  E_t = exp(S_t^T/16)
  Z_t[n] = sum_m E_t[m,n];  O_t^T[c,n] = sum_m Vt^T[m,c] E_t[m,n] / Z_t[n]
  out = Xs + bv + (1/3) sum_t O_t^T
attn.mean(-1) of a softmax is exactly 1/N, so the teacher weights are
uniformly 1/3; folded with 1/Z_t into one reciprocal (ones-vector of 3.0
in the Z row-sum matmul), applied to E_t before the O matmuls so all
teachers accumulate into one PSUM region. Matmuls run in float32r (full
PE rate; plain fp32 takes 2 half-speed passes) with producers rounding
explicitly. Softmax max-subtraction skipped: |S/16| <~ 7 for this regime.

Sharding: data-parallel over batch, B=8 -> one batch element per core.
"""

import sys

sys.path.insert(0, "/opt/trn_rl_repo")

import numpy as np

import concourse.bass as bass
import concourse.tile as tile
from concourse import mybir
from concourse.bass_utils import run_bass_kernel_spmd

B, C, H, W = 8, 256, 32, 32
N = H * W  # 1024
T = 3
P = 128
CC = C // P  # 2 c-chunks
MC = N // P  # 8 m-chunks
NH = N // 512  # 2 n-halves
F32 = mybir.dt.float32
F32R = mybir.dt.float32r
SCALE = C ** -0.5  # 1/16


def build_nc():
    nc = bass.Bass()
    xs_d = nc.dram_tensor("xs", [C, N], F32, kind="ExternalInput")
    xt_d = nc.dram_tensor("xt", [T, C, N], F32, kind="ExternalInput")
    wqT_d = nc.dram_tensor("wqT", [C, C], F32, kind="ExternalInput")
    wkT_d = nc.dram_tensor("wkT", [C, C], F32, kind="ExternalInput")
    wvT_d = nc.dram_tensor("wvT", [C, C], F32, kind="ExternalInput")
    bq_d = nc.dram_tensor("bq", [C, 1], F32, kind="ExternalInput")
    bk_d = nc.dram_tensor("bk", [C, 1], F32, kind="ExternalInput")
    bv_d = nc.dram_tensor("bv", [C, 1], F32, kind="ExternalInput")
    out_d = nc.dram_tensor("out", [C, N], F32, kind="ExternalOutput")

    with tile.TileContext(nc) as tc:
        with (
            tc.tile_pool(name="consts", bufs=1) as consts,
            tc.tile_pool(name="ldpool", bufs=2) as ldpool,
            tc.tile_pool(name="kpool", bufs=6) as kpool,
            tc.tile_pool(name="vpool", bufs=24) as vpool,
            tc.tile_pool(name="epool", bufs=10) as epool,
            tc.tile_pool(name="rpool", bufs=1) as rpool,
            tc.tile_pool(name="bpool", bufs=2) as bpool,
            tc.tile_pool(name="tpool", bufs=2) as tpool,
            tc.tile_pool(name="opool", bufs=2) as opool,
            tc.tile_pool(name="ps", bufs=4, space="PSUM") as ps,
            tc.tile_pool(name="po", bufs=2, space="PSUM") as po,
            tc.tile_pool(name="zps", bufs=2, space="PSUM") as zps,
        ):
            # ---- loads + one-time rounding copies to float32r ----
            def load_r(dram_ap, shape, tag, keep_f32=False, conv_act=False):
                ld = ldpool.tile(shape, F32, tag=f"ld{shape[1]}", name=f"ld_{tag}")
                nc.sync.dma_start(out=ld, in_=dram_ap)
                rt = consts.tile(shape, F32R, tag=tag, name=f"r_{tag}")
                if conv_act:
                    nc.scalar.copy(rt, ld)
                else:
                    nc.vector.tensor_copy(rt, ld)
                if keep_f32:
                    ft = consts.tile(shape, F32, tag=f"f{tag}", name=f"f_{tag}")
                    nc.vector.tensor_copy(ft, ld)
                    return rt, ft
                return rt

            xs_r, xs_sb = [], []
            wqT_r, wkT_r, wvT_r = [], [], []
            bq_sb, bk_sb, bv_sb = [], [], []
            for ci in range(CC):
                sl = slice(ci * P, (ci + 1) * P)
                rt, ft = load_r(xs_d[sl, :], [P, N], f"xs{ci}", keep_f32=True,
                                conv_act=False)
                xs_r.append(rt)
                xs_sb.append(ft)
                wqT_r.append(load_r(wqT_d[sl, :], [P, C], f"wq{ci}"))
                wkT_r.append(load_r(wkT_d[sl, :], [P, C], f"wk{ci}"))
                wvT_r.append(load_r(wvT_d[sl, :], [P, C], f"wv{ci}"))
                for lst, dram, tg in (
                    (bq_sb, bq_d, "bq"), (bk_sb, bk_d, "bk"), (bv_sb, bv_d, "bv"),
                ):
                    b_ = consts.tile([P, 1], F32, tag=f"{tg}{ci}", name=f"{tg}{ci}")
                    nc.sync.dma_start(out=b_, in_=dram[sl, :])
                    lst.append(b_)
            xt_r = [[load_r(xt_d[t, ci * P:(ci + 1) * P, :], [P, N],
                            f"xt{t}{ci}", conv_act=False) for ci in range(CC)]
                    for t in range(T)]
            ones3 = consts.tile([P, 1], F32, tag="ones3", name="ones3")
            nc.vector.memset(ones3, 3.0)
            ones3r = consts.tile([P, 1], F32R, tag="ones3r", name="ones3r")
            nc.vector.tensor_copy(ones3r, ones3)
            ones_row = consts.tile([1, P], F32, tag="ones_row", name="ones_row")
            nc.vector.memset(ones_row, 1.0)
            ones_rowr = consts.tile([1, P], F32R, tag="ones_rowr",
                                    name="ones_rowr")
            nc.vector.tensor_copy(ones_rowr, ones_row)

            # ---- running output accumulator: acc = xs + bv ----
            acc = []
            for co in range(CC):
                a_ = consts.tile([P, N], F32, tag=f"acc{co}", name=f"acc{co}")
                nc.vector.tensor_scalar_add(a_, xs_sb[co], bv_sb[co])
                acc.append(a_)

            # ---- Q projection: Q[c,n] (float32r output for the S matmuls) ----
            q_sb = []
            for co in range(CC):
                qt = consts.tile([P, N], F32R, tag=f"q{co}", name=f"q{co}")
                for nh in range(NH):
                    qp = ps.tile([P, 512], F32, tag="ps", name="qp")
                    for ci in range(CC):
                        nc.tensor.matmul(
                            qp,
                            wqT_r[ci][:, co * P:(co + 1) * P],
                            xs_r[ci][:, nh * 512:(nh + 1) * 512],
                            start=(ci == 0),
                            stop=(ci == CC - 1),
                        )
                    nc.vector.tensor_scalar_add(
                        qt[:, nh * 512:(nh + 1) * 512], qp, bq_sb[co]
                    )
                q_sb.append(qt)

            # ---- all teachers' K and V^T projections up front ----
            k_all, v_all = [], []
            for t in range(T):
                k_sb = []
                for co in range(CC):
                    kt = kpool.tile([P, N], F32R, tag="k", name=f"k{t}{co}")
                    for nh in range(NH):
                        kp = ps.tile([P, 512], F32, tag="ps", name="kp")
                        for ci in range(CC):
                            nc.tensor.matmul(
                                kp,
                                wkT_r[ci][:, co * P:(co + 1) * P],
                                xt_r[t][ci][:, nh * 512:(nh + 1) * 512],
                                start=(ci == 0),
                                stop=(ci == CC - 1),
                            )
                        nc.vector.tensor_scalar_add(
                            kt[:, nh * 512:(nh + 1) * 512], kp, bk_sb[co]
                        )
                    k_sb.append(kt)
                k_all.append(k_sb)
                vT = []
                for mi in range(MC):
                    vp = ps.tile([P, C], F32, tag="ps", name="vp")
                    for ci in range(CC):
                        nc.tensor.matmul(
                            vp,
                            xt_r[t][ci][:, mi * P:(mi + 1) * P],
                            wvT_r[ci],
                            start=(ci == 0),
                            stop=(ci == CC - 1),
                        )
                    vt_ = vpool.tile([P, C], F32R, tag="v", name=f"v{t}{mi}")
                    nc.any.tensor_copy(vt_, vp)
                    vT.append(vt_)
                v_all.append(vT)

            for t in range(T):
                k_sb = k_all[t]
                vT = v_all[t]
                # per-teacher PSUM accumulators: Z rows; O done per c-chunk
                zpt = [zps.tile([1, 512], F32, tag="zp", name=f"zp{t}{nh}")
                       for nh in range(NH)]
                # S^T -> exp(float32r) -> e; Z matmuls consume e directly
                e = []
                for mi in range(MC):
                    et = epool.tile([P, N], F32R, tag="e", name=f"e{t}{mi}")
                    for nh in range(NH):
                        sp = ps.tile([P, 512], F32, tag="ps", name="sp")
                        for ci in range(CC):
                            nc.tensor.matmul(
                                sp,
                                k_sb[ci][:, mi * P:(mi + 1) * P],
                                q_sb[ci][:, nh * 512:(nh + 1) * 512],
                                start=(ci == 0),
                                stop=(ci == CC - 1),
                            )
                        nc.scalar.activation(
                            et[:, nh * 512:(nh + 1) * 512],
                            sp,
                            func=mybir.ActivationFunctionType.Exp,
                            scale=SCALE,
                        )
                    e.append(et)
                    for nh in range(NH):
                        nc.tensor.matmul(
                            zpt[nh], ones3r,
                            et[:, nh * 512:(nh + 1) * 512],
                            start=(mi == 0), stop=(mi == MC - 1),
                        )
                # recipZ = 1/(3 Z); broadcast along partitions via DMA
                recip = rpool.tile([1, N], F32, tag="r", name=f"recip{t}")
                for nh in range(NH):
                    nc.vector.reciprocal(
                        recip[:, nh * 512:(nh + 1) * 512], zpt[nh]
                    )
                recipr = rpool.tile([1, N], F32R, tag="rr", name=f"recipr{t}")
                nc.vector.tensor_copy(recipr, recip)
                bcast = bpool.tile([P, N], F32, tag="b", name=f"bcast{t}")
                for nh in range(NH):
                    bp = ps.tile([P, 512], F32, tag="ps", name="bp")
                    nc.tensor.matmul(
                        bp, ones_rowr, recipr[:, nh * 512:(nh + 1) * 512],
                        start=True, stop=True,
                    )
                    nc.vector.tensor_copy(
                        bcast[:, nh * 512:(nh + 1) * 512], bp)
                # O accumulation per c-chunk, then late normalization:
                # acc += O_t[co] * bcast
                for co in range(CC):
                    otp = [po.tile([P, 512], F32, tag="po", name=f"ot{t}{co}{nh}")
                           for nh in range(NH)]
                    for mi in range(MC):
                        for nh in range(NH):
                            nc.tensor.matmul(
                                otp[nh],
                                vT[mi][:, co * P:(co + 1) * P],
                                e[mi][:, nh * 512:(nh + 1) * 512],
                                start=(mi == 0),
                                stop=(mi == MC - 1),
                            )
                    tmp = tpool.tile([P, N], F32, tag="tmp", name=f"tmp{t}{co}")
                    for nh in range(NH):
                        nc.vector.tensor_mul(
                            tmp[:, nh * 512:(nh + 1) * 512],
                            otp[nh],
                            bcast[:, nh * 512:(nh + 1) * 512],
                        )
                    nc.vector.tensor_add(acc[co], acc[co], tmp)

            # ---- store straight from the accumulators ----
            for co in range(CC):
                nc.sync.dma_start(out=out_d[co * P:(co + 1) * P, :], in_=acc[co])

    _split_multi_waits(nc)
    if not nc.is_finalized():
        nc.finalize()
    return nc


def _split_multi_waits(nc):
    """walrus can encode at most one sync-wait per instruction. Hoist every
    wait of a multi-wait instruction onto single-wait nops on the same
    engine, placed immediately before it in program order."""
    fixes = []
    for fn in nc.m.functions:
        for blk in fn.blocks:
            for inst in blk.instructions:
                si = getattr(inst, "sync_info", None)
                if (si is not None and si.on_wait and len(si.on_wait) > 1
                        and getattr(inst, "engine", None) is not None):
                    fixes.append((blk, inst))
    for blk, inst in fixes:
        si = inst.sync_info
        waits = list(si.on_wait)
        nops = []
        for w in waits:
            nop = nc.engines[inst.engine].nop(nofuse=True).ins
            nop.sync_info = mybir.SyncInfo(on_wait=[w], on_update=[])
            nops.append(nop)
        inst.sync_info = mybir.SyncInfo(on_wait=[], on_update=list(si.on_update))
        nop_names = {n.name for n in nops}
        for fn2 in nc.m.functions:
            for blk2 in fn2.blocks:
                blk2.instructions = [
                    i for i in blk2.instructions if i.name not in nop_names
                ]
        pos = next(i for i, x in enumerate(blk.instructions)
                   if x.name == inst.name)
        blk.instructions = (blk.instructions[:pos] + nops
                            + blk.instructions[pos:])


_NC = None


def _get_nc():
    global _NC
    if _NC is None:
        _NC = build_nc()
    return _NC


def make_in_maps(student_feat, t_feat0, t_feat1, t_feat2,
                 Wq, bq, Wk, bk, Wv, bv):
    xs = np.ascontiguousarray(student_feat.reshape(B, C, N), dtype=np.float32)
    xt = np.ascontiguousarray(
        np.stack([t_feat0, t_feat1, t_feat2], axis=1).reshape(B, T, C, N),
        dtype=np.float32)
    wqT = np.ascontiguousarray(Wq.T, dtype=np.float32)
    wkT = np.ascontiguousarray(Wk.T, dtype=np.float32)
    wvT = np.ascontiguousarray(Wv.T, dtype=np.float32)
    bqc = np.ascontiguousarray(bq.reshape(C, 1), dtype=np.float32)
    bkc = np.ascontiguousarray(bk.reshape(C, 1), dtype=np.float32)
    bvc = np.ascontiguousarray(bv.reshape(C, 1), dtype=np.float32)
    return [
        {"xs": xs[b], "xt": xt[b], "wqT": wqT, "wkT": wkT, "wvT": wvT,
         "bq": bqc, "bk": bkc, "bv": bvc}
        for b in range(B)
    ]


def run(in_maps, trace=False):
    nc = _get_nc()
    return run_bass_kernel_spmd(nc, in_maps, core_ids=list(range(B)),
                                trace=trace)


def kernel(student_feat, t_feat0, t_feat1, t_feat2,
           Wq, bq, Wk, bk, Wv, bv):
    in_maps = make_in_maps(student_feat, t_feat0, t_feat1, t_feat2,
                           Wq, bq, Wk, bk, Wv, bv)
    res = run(in_maps, trace=False)
    out = np.stack([res.results[b]["out"].reshape(C, H, W) for b in range(B)])
    return out.astype(np.float32)
